# revision 6
# baseline (speedup 1.0000x reference)
"""Trainium2 Bass kernel for nn_CompILESegmenter (gated GRU segmenter).

Strategy
--------
The hard gate g_hard(b,t) = [sigmoid(MLP(e_t)) > 0.5] depends only on the
input e, not on the recurrent state h, and when it fires h is reset to 0.
So each batch row's T=512 sequence splits into many independent short
chains (mean length ~2 for this data distribution).  We compute the gate
on the host (cheap: 2% of FLOPs), cut the (b,t) grid into chains, and
greedily pack the chains of each core's batch shard into 128 SIMD "lanes".
The device then runs the recurrence over ~T_pack = ceil(B/8*T/128) packed
iterations instead of T sequential steps, with all 128 PE stationary
columns busy (vs 8 for plain data parallelism).

Per packed iteration i (all [128 lanes, .] tiles):
    rz  = e_i @ Wih[rz].T + h_i @ Whh[rz].T + b      (PSUM accumulation)
    i_n = e_i @ Wih[n].T + b_ihn ;  h_n = h_i @ Whh[n].T + b_hhn
    r = sig(rz_r); zc = sig(-rz_z); n = tanh(i_n + r*h_n)
    h' = hr + zc*(n - hr)            (hr = keep_i * h_prev, keep=0 at chain starts)
    mu,lv = h' @ [mu_w;lv_w].T ; z = mu + exp(lv/2)*noise_i
Matmuls run in bf16 (fp32 PSUM accumulation); elementwise math in fp32.

Sharding: data-parallel over B (8 rows/core, replicated weights), with the
chain-packing applied independently per core.  No collectives.
"""

import numpy as np
import ml_dtypes
from contextlib import ExitStack

B, T, DIM, ZDIM, GH = 64, 512, 1024, 128, 64
NCORES, BPC, LANES, KT = 8, 8, 128, 8
BF16 = ml_dtypes.bfloat16

_PROG_CACHE = {}
_LAST_RUN = None   # (nc, in_maps) of the most recent kernel() call, for benching
_LAST_L = [0]


# ----------------------------------------------------------------------------
# Host-side schedule
# ----------------------------------------------------------------------------

def _pack_core(g_hard_core):
    """g_hard_core: (BPC, T) bool. Returns lane chain lists [(flat_start, len)]."""
    chains = []
    for bl in range(BPC):
        gh = g_hard_core[bl]
        starts = np.flatnonzero(gh).tolist()
        if not starts or starts[0] != 0:
            starts = [0] + starts
        for j, s in enumerate(starts):
            e = starts[j + 1] if j + 1 < len(starts) else T
            chains.append((bl * T + s, e - s))
    lens = np.array([c[1] for c in chains])
    order = np.argsort(-lens, kind="stable")
    lanes = [[] for _ in range(LANES)]
    loads = np.zeros(LANES, np.int64)
    for ci in order:
        l = int(loads.argmin())
        lanes[l].append(chains[ci])
        loads[l] += chains[ci][1]
    return lanes, int(loads.max())


def _build_schedule(g_hard):
    """Returns (L, [per-core dict of sched/keep arrays])."""
    packed = []
    L = 1
    for core in range(NCORES):
        lanes, lmax = _pack_core(g_hard[core * BPC:(core + 1) * BPC])
        packed.append(lanes)
        L = max(L, lmax)
    cores = []
    for core in range(NCORES):
        sched = np.full((L, LANES), -1, np.int64)
        keep = np.zeros((L, LANES), np.float32)
        for l, chs in enumerate(packed[core]):
            i = 0
            for (s, ln) in chs:
                for k in range(ln):
                    sched[i, l] = s + k
                    keep[i, l] = 0.0 if k == 0 else 1.0
                    i += 1
        cores.append({"sched": sched, "keep": keep})
    return L, cores


# ----------------------------------------------------------------------------
# Device program
# ----------------------------------------------------------------------------

def _build_program(L, b2):
    import concourse.bass as bass  # noqa: F401
    import concourse.tile as tile
    from concourse import bacc, mybir

    dt = mybir.dt
    AF = mybir.ActivationFunctionType
    nc = bacc.Bacc("TRN2", target_bir_lowering=False, debug=False)

    def din(name, shape, dty=dt.bfloat16):
        return nc.dram_tensor(name, shape, dty, kind="ExternalInput").ap()

    def dout(name, shape, dty=dt.float32):
        return nc.dram_tensor(name, shape, dty, kind="ExternalOutput").ap()

    eT = din("eT", [L, 128, KT, LANES])
    nzD = din("nz", [L, LANES, ZDIM], dt.float32)
    keepD = din("keep", [LANES, L], dt.float32)
    keepT = din("keepT", [L, 128, KT * LANES])
    wihD = din("wih", [128, KT, 3 * DIM])
    whhD = din("whh", [128, KT, 3 * DIM])
    hwD = din("hw", [128, KT, 2 * ZDIM])
    gw1D = din("gw1", [128, KT, GH])
    gw2D = din("gw2", [128, GH], dt.float32)
    brzD = din("brz", [1, 2 * DIM])
    binD = din("bin", [1, DIM])
    bhnD = din("bhn", [1, DIM])
    bhdD = din("bhd", [1, 2 * ZDIM])
    bg1D = din("bg1", [1, GH])
    Z = dout("Z", [L, LANES, ZDIM])
    MU = dout("MU", [L, LANES, ZDIM])
    LV = dout("LV", [L, LANES, ZDIM])
    G = dout("G", [LANES, L])

    with tile.TileContext(nc) as tc, ExitStack() as ctx:
        const = ctx.enter_context(tc.tile_pool(name="const", bufs=1))
        io2 = ctx.enter_context(tc.tile_pool(name="io", bufs=3))
        st = ctx.enter_context(tc.tile_pool(name="state", bufs=2))
        vt = ctx.enter_context(tc.tile_pool(name="vtmp", bufs=2))
        ot = ctx.enter_context(tc.tile_pool(name="outs", bufs=3))
        ps = ctx.enter_context(tc.tile_pool(name="psum", bufs=8, space="PSUM"))

        def load_const(nm, ap, shape, dty):
            t = const.tile(shape, dty, name=nm, tag=nm)
            nc.sync.dma_start(t[:], ap)
            return t

        wih = load_const("c_wih", wihD, [128, KT, 3 * DIM], dt.bfloat16)
        whh = load_const("c_whh", whhD, [128, KT, 3 * DIM], dt.bfloat16)
        hw = load_const("c_hw", hwD, [128, KT, 2 * ZDIM], dt.bfloat16)
        gw1 = load_const("c_gw1", gw1D, [128, KT, GH], dt.bfloat16)
        gw2 = load_const("c_gw2", gw2D, [128, GH], dt.float32)
        keep = load_const("c_keep", keepD, [LANES, L], dt.float32)
        brz = load_const("c_brz", brzD, [1, 2 * DIM], dt.bfloat16)
        bin_ = load_const("c_bin", binD, [1, DIM], dt.bfloat16)
        bhn = load_const("c_bhn", bhnD, [1, DIM], dt.bfloat16)
        bhd = load_const("c_bhd", bhdD, [1, 2 * ZDIM], dt.bfloat16)
        bg1 = load_const("c_bg1", bg1D, [1, GH], dt.bfloat16)

        ones = const.tile([1, 128], dt.bfloat16)
        nc.vector.memset(ones[:], 1.0)
        g_acc = const.tile([LANES, L], dt.float32)
        h0 = const.tile([LANES, DIM], dt.float32)
        nc.vector.memset(h0[:], 0.0)

        h_prev = h0
        hT_prev = None

        for i in range(L):
            e_sb = io2.tile([128, KT, LANES], dt.bfloat16, tag="e")
            nc.sync.dma_start(e_sb[:], eT[i])
            nz_sb = io2.tile([LANES, ZDIM], dt.float32, tag="nz")
            nc.sync.dma_start(nz_sb[:], nzD[i])
            if i < L - 1:
                kT_sb = io2.tile([128, KT * LANES], dt.bfloat16, tag="kT")
                nc.sync.dma_start(kT_sb[:], keepT[i])

            rz_ps = [ps.tile([LANES, 512], dt.float32, tag="ps", name=f"rz{i}_{c}") for c in range(4)]
            in_ps = [ps.tile([LANES, 512], dt.float32, tag="ps", name=f"in{i}_{c}") for c in range(2)]
            hn_ps = [ps.tile([LANES, 512], dt.float32, tag="ps", name=f"hn{i}_{c}") for c in range(2)]

            # ---- e contribution ----
            for k in range(KT):
                stat = e_sb[:, k, :]
                for c in range(4):
                    nc.tensor.matmul(rz_ps[c][:], stat, wih[:, k, 512 * c:512 * (c + 1)],
                                     start=(k == 0), stop=False)
                for c in range(2):
                    nc.tensor.matmul(in_ps[c][:], stat, wih[:, k, 2048 + 512 * c:2048 + 512 * (c + 1)],
                                     start=(k == 0), stop=False)
            # bias rows
            for c in range(4):
                nc.tensor.matmul(rz_ps[c][:], ones[:], brz[:, 512 * c:512 * (c + 1)],
                                 start=False, stop=(i == 0))
            for c in range(2):
                nc.tensor.matmul(in_ps[c][:], ones[:], bin_[:, 512 * c:512 * (c + 1)],
                                 start=False, stop=True)
            for c in range(2):
                nc.tensor.matmul(hn_ps[c][:], ones[:], bhn[:, 512 * c:512 * (c + 1)],
                                 start=True, stop=(i == 0))

            # ---- h contribution ----
            if i > 0:
                for k in range(KT):
                    stat = hT_prev[:, k, :]
                    for c in range(4):
                        nc.tensor.matmul(rz_ps[c][:], stat, whh[:, k, 512 * c:512 * (c + 1)],
                                         start=False, stop=(k == KT - 1))
                    for c in range(2):
                        nc.tensor.matmul(hn_ps[c][:], stat, whh[:, k, 2048 + 512 * c:2048 + 512 * (c + 1)],
                                         start=False, stop=(k == KT - 1))

            # ---- recurrent elementwise ----
            if i == 0:
                hr = h0
            else:
                hr = st.tile([LANES, DIM], dt.float32, tag="hr")
                nc.vector.tensor_scalar_mul(hr[:], h_prev[:], keep[:, i:i + 1])
            h_new = st.tile([LANES, DIM], dt.float32, tag="h")
            for hf in range(2):
                sl = slice(512 * hf, 512 * (hf + 1))
                r_h = vt.tile([LANES, 512], dt.float32, tag="r")
                nc.scalar.activation(r_h[:], rz_ps[hf][:], AF.Sigmoid)
                zc_h = vt.tile([LANES, 512], dt.float32, tag="zc")
                nc.scalar.activation(zc_h[:], rz_ps[2 + hf][:], AF.Sigmoid, scale=-1.0)
                t1 = vt.tile([LANES, 512], dt.float32, tag="t1")
                nc.vector.tensor_mul(t1[:], r_h[:], hn_ps[hf][:])
                t2 = vt.tile([LANES, 512], dt.float32, tag="t2")
                nc.vector.tensor_add(t2[:], t1[:], in_ps[hf][:])
                n_h = vt.tile([LANES, 512], dt.float32, tag="n")
                nc.scalar.activation(n_h[:], t2[:], AF.Tanh)
                d_h = vt.tile([LANES, 512], dt.float32, tag="d")
                nc.vector.tensor_sub(d_h[:], n_h[:], hr[:, sl])
                e2 = vt.tile([LANES, 512], dt.float32, tag="e2")
                nc.vector.tensor_mul(e2[:], zc_h[:], d_h[:])
                nc.vector.tensor_add(h_new[:, sl], hr[:, sl], e2[:])

            # ---- gate MLP (uses freed PSUM slots) ----
            g_ps = ps.tile([LANES, GH], dt.float32, tag="ps")
            for k in range(KT):
                nc.tensor.matmul(g_ps[:], e_sb[:, k, :], gw1[:, k, :],
                                 start=(k == 0), stop=False)
            nc.tensor.matmul(g_ps[:], ones[:], bg1[:], start=False, stop=True)
            t1g = vt.tile([LANES, GH], dt.float32, tag="t1g")
            nc.scalar.activation(t1g[:], g_ps[:], AF.Tanh)
            qg = vt.tile([LANES, GH], dt.float32, tag="qg")
            nc.vector.tensor_mul(qg[:], t1g[:], gw2[:])
            lg = vt.tile([LANES, 1], dt.float32, tag="lg")
            nc.vector.tensor_reduce(lg[:], qg[:], axis=mybir.AxisListType.X,
                                    op=mybir.AluOpType.add)
            if b2 != 0.0:
                lg2 = vt.tile([LANES, 1], dt.float32, tag="lg2")
                nc.vector.tensor_scalar_add(lg2[:], lg[:], float(b2))
                lg = lg2
            nc.scalar.activation(g_acc[:, i:i + 1], lg[:], AF.Sigmoid)

            # ---- transpose h' for matmuls ----
            hbf = vt.tile([LANES, DIM], dt.bfloat16, tag="hbf")
            nc.vector.tensor_copy(hbf[:], h_new[:])
            hT_raw = st.tile([128, KT, LANES], dt.bfloat16, tag="hTr")
            for k in range(KT):
                nc.sync.dma_start_transpose(hT_raw[:, k, :], hbf[:, 128 * k:128 * (k + 1)])

            # ---- heads ----
            hd_ps = ps.tile([LANES, 2 * ZDIM], dt.float32, tag="ps")
            for k in range(KT):
                nc.tensor.matmul(hd_ps[:], hT_raw[:, k, :], hw[:, k, :],
                                 start=(k == 0), stop=False)
            nc.tensor.matmul(hd_ps[:], ones[:], bhd[:], start=False, stop=True)

            # ---- apply next-step reset to transposed h ----
            if i < L - 1:
                hT_rdy = st.tile([128, KT, LANES], dt.bfloat16, tag="hTd")
                nc.vector.tensor_mul(hT_rdy[:], hT_raw[:], kT_sb[:])
            else:
                hT_rdy = None

            # ---- z / mu / lv ----
            sc = vt.tile([LANES, ZDIM], dt.float32, tag="sc")
            nc.scalar.activation(sc[:], hd_ps[:, ZDIM:], AF.Exp, scale=0.5)
            zt = ot.tile([LANES, ZDIM], dt.float32, tag="zt")
            nc.vector.tensor_mul(zt[:], sc[:], nz_sb[:])
            zt2 = ot.tile([LANES, ZDIM], dt.float32, tag="zt2")
            nc.vector.tensor_add(zt2[:], zt[:], hd_ps[:, :ZDIM])
            mu_sb = ot.tile([LANES, ZDIM], dt.float32, tag="mu")
            nc.scalar.copy(mu_sb[:], hd_ps[:, :ZDIM])
            lv_sb = ot.tile([LANES, ZDIM], dt.float32, tag="lv")
            nc.scalar.copy(lv_sb[:], hd_ps[:, ZDIM:])
            nc.sync.dma_start(Z[i], zt2[:])
            nc.sync.dma_start(MU[i], mu_sb[:])
            nc.sync.dma_start(LV[i], lv_sb[:])

            h_prev, hT_prev = h_new, hT_rdy

        nc.sync.dma_start(G[:], g_acc[:])

    nc.compile()
    return nc


def _get_program(L, b2):
    key = (L, float(b2))
    if key not in _PROG_CACHE:
        _PROG_CACHE[key] = _build_program(L, b2)
    return _PROG_CACHE[key]


# ----------------------------------------------------------------------------
# kernel()
# ----------------------------------------------------------------------------

def kernel(e_seq, noise, gate_w1, gate_b1, gate_w2, gate_b2,
           w_ih, b_ih, w_hh, b_hh, mu_w, mu_b, lv_w, lv_b):
    from concourse.bass_utils import run_bass_kernel_spmd

    e = np.ascontiguousarray(np.asarray(e_seq, np.float32))
    nz = np.ascontiguousarray(np.asarray(noise, np.float32))
    gw1 = np.asarray(gate_w1, np.float32)
    gb1 = np.asarray(gate_b1, np.float32)
    gw2 = np.asarray(gate_w2, np.float32)
    gb2 = np.asarray(gate_b2, np.float32)
    wih = np.asarray(w_ih, np.float32)
    bih = np.asarray(b_ih, np.float32)
    whh = np.asarray(w_hh, np.float32)
    bhh = np.asarray(b_hh, np.float32)
    muw = np.asarray(mu_w, np.float32)
    mub = np.asarray(mu_b, np.float32)
    lvw = np.asarray(lv_w, np.float32)
    lvb = np.asarray(lv_b, np.float32)

    # --- host gate (fp32) for the schedule ---
    ef = e.reshape(B * T, DIM)
    a1 = np.tanh(ef @ gw1.T + gb1)
    logit = (a1 @ gw2.T)[:, 0] + gb2[0]
    g_hard = (logit > 0.0).reshape(B, T)

    L, cores = _build_schedule(g_hard)

    # --- shared (replicated) device weights ---
    def t8(w):  # (N, DIM) -> (128, KT, N) stationary-transposed layout
        return np.ascontiguousarray(
            w.T.reshape(KT, 128, w.shape[0]).transpose(1, 0, 2))

    wih_d = t8(wih).astype(BF16)
    whh_d = t8(whh).astype(BF16)
    hw_d = t8(np.concatenate([muw, lvw], 0)).astype(BF16)
    gw1_d = t8(gw1).astype(BF16)
    gw2_d = np.ascontiguousarray(np.broadcast_to(gw2[0], (128, GH))).astype(np.float32)
    brz_d = (bih + bhh)[None, :2 * DIM].astype(BF16)
    bin_d = bih[None, 2 * DIM:].astype(BF16)
    bhn_d = bhh[None, 2 * DIM:].astype(BF16)
    bhd_d = np.concatenate([mub, lvb])[None, :].astype(BF16)
    bg1_d = gb1[None, :].astype(BF16)

    shared = dict(wih=wih_d, whh=whh_d, hw=hw_d, gw1=gw1_d, gw2=gw2_d,
                  brz=brz_d, bin=bin_d, bhn=bhn_d, bhd=bhd_d, bg1=bg1_d)

    # --- per-core packed inputs ---
    in_maps = []
    for core in range(NCORES):
        sch = cores[core]["sched"]          # (L, LANES)
        keep = cores[core]["keep"]          # (L, LANES)
        idx = np.where(sch >= 0, sch, 0)
        e_rows = e[core * BPC:(core + 1) * BPC].reshape(BPC * T, DIM)
        nz_rows = nz[core * BPC:(core + 1) * BPC].reshape(BPC * T, ZDIM)
        ep = e_rows[idx.reshape(-1)].reshape(L, LANES, KT, 128)
        eTp = np.ascontiguousarray(ep.transpose(0, 3, 2, 1)).astype(BF16)
        nzp = np.ascontiguousarray(nz_rows[idx.reshape(-1)].reshape(L, LANES, ZDIM))
        keep_d = np.ascontiguousarray(keep.T)  # (LANES, L)
        kn = np.zeros((L, LANES), np.float32)
        kn[:L - 1] = keep[1:]
        keepT_d = np.ascontiguousarray(
            np.broadcast_to(kn[:, None, None, :], (L, 128, KT, LANES))
            .reshape(L, 128, KT * LANES)).astype(BF16)
        in_maps.append(dict(eT=eTp, nz=nzp, keep=keep_d, keepT=keepT_d, **shared))

    nc = _get_program(L, gb2[0])
    global _LAST_RUN
    _LAST_RUN = (nc, in_maps)
    _LAST_L[0] = L
    res = run_bass_kernel_spmd(nc, in_maps, list(range(NCORES)))

    # --- unpack ---
    z = np.zeros((B, T, ZDIM), np.float32)
    mu = np.zeros((B, T, ZDIM), np.float32)
    lv = np.zeros((B, T, ZDIM), np.float32)
    g = np.zeros((B, T, 1), np.float32)
    for core in range(NCORES):
        r = res.results[core]
        sch = cores[core]["sched"]
        valid = sch >= 0
        flat = sch[valid]
        bs = core * BPC + flat // T
        ts = flat % T
        z[bs, ts] = r["Z"].reshape(L * LANES, ZDIM)[valid.reshape(-1)]
        mu[bs, ts] = r["MU"].reshape(L * LANES, ZDIM)[valid.reshape(-1)]
        lv[bs, ts] = r["LV"].reshape(L * LANES, ZDIM)[valid.reshape(-1)]
        g[bs, ts, 0] = r["G"].T.reshape(-1)[valid.reshape(-1)]
    return z, g, mu, lv
